# revision 1
# baseline (speedup 1.0000x reference)
"""AttentionPooling Trainium2 Bass kernel.

Problem (hardcoded shapes): B=64, T=4096, D=256, U=64
    uit    = tanh(inputs @ w + b)          # [B,T,U]
    scores = (uit @ u)[..., 0]             # [B,T]
    scores += (1-mask) * -1e9
    attn   = softmax(scores, axis=1)       # [B,T]
    out    = einsum('btd,bt->bd', inputs, attn)

Sharding: pure data-parallel, 8 examples per core across 8 NeuronCores.

Per-core design (EX=8 examples):
  - x loaded HBM->SBUF with f32->bf16 cast (SWDGE), natural layout
    x2[p, h, j, d'] = x[128j+p, 128h+d']   (t=128j+p, d=128h+d')
  - xbar DMA transpose (bf16) gives xT_h[d', j, t'] = x[128j+t', 128h+d']
  - comp1 (PE):   z^T tiles = w_h^T @ x^T_h accumulated over h (D-chunks),
                  packed 2 T-groups per PSUM tile via output partition halves
  - tanh (ACT):   uitT = tanh(z + b) with per-partition bias, bf16 out
  - comp2 (PE):   scores via uitT chunks as stationary against block-diag u,
                  landing scores^T in PSUM as [128(t mod 128), 32(t div 128)]
  - softmax:      exp on ACT; mask-mult + row partial sums fused on DVE;
                  full denom via ones-matmul on PE; reciprocal on DVE
  - comp4 (PE):   context = sum_t e_t * x_t with e columns as stationary,
                  x2 tiles as moving operand, PSUM accumulation over T-chunks
  - final (ACT):  context * (1/denom), f32 out -> DMA to HBM
"""

import numpy as np

B, T, D, U = 64, 4096, 256, 64
NCORES = 8
EX = B // NCORES  # 8 examples per core
NJ = T // 128     # 32 T-chunks
NEG_BIG = -1e9

_CACHE = {}


def _build(groups=(4, 4), xp_bufs=6, xtp_bufs=3, tr_split=1, phase_deps=True, pu_bufs=2, pcx_bufs=2, sp_bufs=3, up_bufs=4):
    """Build and compile the per-core Bass program. Returns (nc, names)."""
    import concourse.bacc as bacc
    import concourse.tile as tile
    import concourse.mybir as mybir
    from concourse._compat import axon_active

    f32 = mybir.dt.float32
    bf16 = mybir.dt.bfloat16
    i32 = mybir.dt.int32
    AF = mybir.ActivationFunctionType
    ALU = mybir.AluOpType

    nc = bacc.Bacc("TRN2", target_bir_lowering=False, debug=not axon_active())

    x_d = nc.dram_tensor("x", (EX, T, D), f32, kind="ExternalInput").ap()
    mask_d = nc.dram_tensor("mask", (EX, T), i32, kind="ExternalInput").ap()
    w_d = nc.dram_tensor("w", (D, U), f32, kind="ExternalInput").ap()
    b_d = nc.dram_tensor("b", (U,), f32, kind="ExternalInput").ap()
    u_d = nc.dram_tensor("u", (U, 1), f32, kind="ExternalInput").ap()
    out_d = nc.dram_tensor("out", (EX, D), f32, kind="ExternalOutput").ap()

    from concourse.tile import add_dep_helper

    with tile.TileContext(nc) as tc:
        with (
            tc.tile_pool(name="consts", bufs=1) as cp,
            tc.tile_pool(name="xp", bufs=xp_bufs) as xp,
            tc.tile_pool(name="xtp", bufs=xtp_bufs) as xtp,
            tc.tile_pool(name="up", bufs=up_bufs) as up,
            tc.tile_pool(name="sp", bufs=sp_bufs) as sp,
            tc.tile_pool(name="pu", bufs=pu_bufs, space="PSUM") as pu,
            tc.tile_pool(name="psa", bufs=2, space="PSUM") as psa,
            tc.tile_pool(name="psb", bufs=2, space="PSUM") as psb,
            tc.tile_pool(name="pcx", bufs=pcx_bufs, space="PSUM") as pcx,
        ):
            # ---- constants + all masks upfront (copy-mode DMAs) ----

            C = {}
            def emit_consts():
                const_copies = []
                w_bf = cp.tile([128, 2, U], bf16, tag="w")
                const_copies.append(
                    nc.gpsimd.dma_start(
                        out=w_bf, in_=w_d.rearrange("(c p) u -> p c u", p=128)
                    )
                )
                b_sb = cp.tile([128, 1], f32, tag="b")
                b_2d = b_d.rearrange("(u o) -> u o", o=1)
                const_copies.append(nc.sync.dma_start(out=b_sb[0:U, :], in_=b_2d))
                const_copies.append(nc.sync.dma_start(out=b_sb[U:128, :], in_=b_2d))
                u_bd = cp.tile([128, 2], bf16, tag="u")
                nc.vector.memset(u_bd, 0.0)
                const_copies.append(nc.gpsimd.dma_start(out=u_bd[0:U, 0:1], in_=u_d))
                const_copies.append(nc.gpsimd.dma_start(out=u_bd[U:128, 1:2], in_=u_d))
                ones_b = cp.tile([128, 1], bf16, tag="ones")
                nc.vector.memset(ones_b, 1.0)
                # mask: contiguous load [2][128,128] i32, cast to bf16, xbar
                # transpose -> mT[L][p, q] with q = 32*(ex%4)+j; ex group L=ex//4
                mask3 = mask_d.rearrange("e t -> (e t)").rearrange(
                    "(L q p) -> L q p", L=2, q=128
                )
                mtb = []
                mT = []
                for L in range(2):
                    mi_ = cp.tile([128, 128], i32, tag=f"mi{L}")
                    const_copies.append(nc.sync.dma_start(out=mi_, in_=mask3[L]))
                    mb_ = cp.tile([128, 128], bf16, tag=f"mtb{L}")
                    nc.vector.tensor_copy(out=mb_, in_=mi_)
                    mtb.append(mb_)
                    mT_ = cp.tile([128, 128], bf16, tag=f"mT{L}")
                    mT.append(mT_)
                ctx_all = cp.tile([1, EX * D], f32, tag="ctx")
                C.update(w_bf=w_bf, b_sb=b_sb, u_bd=u_bd, ones_b=ones_b,
                         mT=mT, mtb=mtb, ctx_all=ctx_all,
                         const_copies=const_copies)

            x2s = {}
            xTs = {}
            ps_map = {}
            load_insts = []
            tr_insts = []

            def load(ex):
                x2 = xp.tile([128, 2, NJ, 128], bf16, tag="x2")
                for h in range(2):
                    i_ = nc.gpsimd.dma_start(
                        out=x2[:, h],
                        in_=x_d[ex, :, 128 * h : 128 * (h + 1)].rearrange(
                            "(j p) d -> p j d", p=128
                        ),
                    )
                    load_insts.append(i_)
                x2s[ex] = x2

            def transpose(ex):
                # xTf[p', k, t'] with k = 32*h + j (h j blocked layout)
                if tr_split == 1:
                    t_ = xtp.tile([128, 2 * NJ, 128], bf16, tag="xT")
                    x2flat = x2s[ex].rearrange("p h j d -> p (h j d)")
                    i_ = nc.sync.dma_start(out=t_, in_=x2flat, transpose=True)
                    tr_insts.append(i_)
                    xTs[ex] = t_
                elif tr_split == 2:
                    t_ = xtp.tile([128, 2, NJ, 128], bf16, tag="xT")
                    for h in range(2):
                        i_ = nc.sync.dma_start(
                            out=t_[:, h], in_=x2s[ex][:, h], transpose=True
                        )
                        tr_insts.append(i_)
                    xTs[ex] = t_.rearrange("p h j d -> p (h j) d")
                elif tr_split == 4:
                    # quarter transposes (T-half x D-half): comp1 of T-half H
                    # only needs the two h-quarters of that H
                    t_ = xtp.tile([128, 2, 2, NJ // 2, 128], bf16, tag="xT")
                    for H in range(2):
                        for h in range(2):
                            i_ = nc.sync.dma_start(
                                out=t_[:, h, H],
                                in_=x2s[ex][:, h, 16 * H : 16 * H + 16, :],
                                transpose=True,
                            )
                            tr_insts.append(i_)
                    xTs[ex] = t_.rearrange("p h H j d -> p (h H j) d")
                else:
                    # eighth transposes
                    t_ = xtp.tile([128, 2, 2, 2, NJ // 4, 128], bf16, tag="xT")
                    for H in range(2):
                        for h in range(2):
                            for q in range(2):
                                i_ = nc.sync.dma_start(
                                    out=t_[:, h, H, q],
                                    in_=x2s[ex][
                                        :, h, 16 * H + 8 * q : 16 * H + 8 * q + 8, :
                                    ],
                                    transpose=True,
                                )
                                tr_insts.append(i_)
                    xTs[ex] = t_.rearrange("p h H q j d -> p (h H q j) d")

            def compute(ex):
                x2, xTf = x2s[ex], xTs.pop(ex)
                w_bf, b_sb, u_bd = C["w_bf"], C["b_sb"], C["u_bd"]
                # comp1 + tanh per T-quarter (1-bank pz tiles); comp2 of half
                # H lands in its own PSUM bank so the softmax can run per
                # half without bank-hazard serialization against comp2-H1
                ps_a = psa.tile([128, 17], f32, tag="psa")
                ps_b = psb.tile([128, 16], f32, tag="psb")
                for Q in range(4):
                    H, gp = Q // 2, Q % 2
                    pz = pu.tile([128, 512], f32, tag="pz")
                    for pi in range(2):
                        g = 2 * gp + pi
                        for h in range(2):
                            nc.tensor.matmul(
                                out=pz[64 * pi : 64 * pi + 64, :],
                                lhsT=w_bf[:, h, :],
                                rhs=xTf[
                                    :,
                                    32 * h + 16 * H + 4 * g : 32 * h + 16 * H + 4 * g + 4,
                                    :,
                                ],
                                start=(h == 0),
                                stop=(h == 1),
                            )
                    uitQ = up.tile([128, 512], bf16, tag="uit")
                    nc.scalar.activation(
                        out=uitQ,
                        in_=pz,
                        func=AF.Tanh,
                        bias=b_sb,
                        scale=1.0,
                    )
                    psx = ps_a if H == 0 else ps_b
                    for cq in range(4):
                        j0 = 8 * gp + cq
                        nc.tensor.matmul(
                            out=psx[:, j0 : j0 + 5 : 4],
                            lhsT=uitQ[:, 128 * cq : 128 * cq + 128],
                            rhs=u_bd,
                            start=True,
                            stop=True,
                        )

                ps_map[ex] = (ps_a, ps_b)

            def compute_b(ex):
                x2 = x2s[ex]
                ps_a, ps_b = ps_map.pop(ex)
                ones_b, mT, ctx_all = C["ones_b"], C["mT"], C["ctx_all"]
                # softmax per T-half; comp4's first 16 chunks start while the
                # second half is still in comp2/exp
                q0 = 32 * (ex % 4)
                e_h = []
                e1_h = []
                for Hh, psx in ((0, ps_a), (1, ps_b)):
                    es = sp.tile([128, 16], bf16, tag=f"es{Hh}")
                    nc.scalar.activation(out=es, in_=psx[:, 0:16], func=AF.Exp)
                    e = sp.tile([128, 16], bf16, tag=f"e{Hh}")
                    nc.vector.tensor_mul(
                        e, es, mT[ex // 4][:, q0 + 16 * Hh : q0 + 16 * Hh + 16]
                    )
                    e1f = sp.tile([128, 1], f32, tag=f"e1f{Hh}")
                    nc.vector.tensor_reduce(
                        out=e1f, in_=e, axis=mybir.AxisListType.X, op=ALU.add
                    )
                    e_h.append(e)
                    e1_h.append(e1f)
                e1s = sp.tile([128, 1], f32, tag="e1s")
                nc.vector.tensor_add(e1s, e1_h[0], e1_h[1])
                e1b = sp.tile([128, 1], bf16, tag="e1b")
                nc.vector.tensor_copy(out=e1b, in_=e1s)
                nc.tensor.matmul(
                    out=ps_a[0:1, 16:17], lhsT=e1b, rhs=ones_b, start=True, stop=True
                )
                r_ = sp.tile([1, 1], f32, tag="r")
                nc.vector.reciprocal(out=r_, in_=ps_a[0:1, 16:17])

                # comp4: context accumulation over T-chunks
                pc_ = pcx.tile([1, D], f32, tag="pc")
                for j in range(NJ):
                    nc.tensor.matmul(
                        out=pc_,
                        lhsT=e_h[j // 16][:, j % 16 : j % 16 + 1],
                        rhs=x2[:, :, j, :],
                        start=(j == 0),
                        stop=(j == NJ - 1),
                    )
                del x2s[ex]
                nc.scalar.activation(
                    out=ctx_all[0:1, D * ex : D * ex + D],
                    in_=pc_,
                    func=AF.Copy,
                    scale=r_[0:1, 0:1],
                )

            prev_trs = []
            ex0 = 0
            for grp, gsz in enumerate(groups):
                exs = list(range(ex0, ex0 + gsz))
                ex0 += gsz
                load_insts.clear()
                for ex in exs:
                    load(ex)
                if grp == 0:
                    emit_consts()
                # loads of this group run after the previous group's transposes
                if phase_deps:
                    for li in load_insts:
                        for ti in prev_trs:
                            add_dep_helper(
                                li.ins,
                                ti.ins,
                                reason="phase: loads after prev transposes",
                            )
                tr_insts.clear()
                for ex in exs:
                    transpose(ex)
                if grp == 0:
                    # mask transposes join the first transpose phase
                    for L in range(2):
                        ti_ = nc.sync.dma_start(
                            out=C["mT"][L], in_=C["mtb"][L], transpose=True
                        )
                        tr_insts.append(ti_)
                # transposes run after all loads of this group (and, for the
                # first group, after every constant copy-DMA as well — no
                # copy may be in flight while the xbar is in transpose mode)
                if phase_deps:
                    gate = load_insts + (C["const_copies"] if grp == 0 else [])
                    for ti in tr_insts:
                        for li in gate:
                            add_dep_helper(
                                ti.ins,
                                li.ins,
                                reason="phase: transposes after loads",
                            )
                prev_trs = list(tr_insts)
                for ex in exs:
                    compute(ex)
                for ex in exs:
                    compute_b(ex)

            nc.sync.dma_start(out=out_d.rearrange("e d -> (e d)"), in_=C["ctx_all"])

    nc.compile()
    return nc


def _get_nc(**kw):
    key = tuple(sorted(kw.items()))
    if key not in _CACHE:
        _CACHE[key] = _build(**kw)
    return _CACHE[key]


# phase_deps=True keeps xbar-transpose DMAs strictly serialized against
# copy DMAs (semaphore-enforced phases). Free scheduling (phase_deps=False)
# intermittently hit NRT_EXEC_UNIT_UNRECOVERABLE on hardware — consistent
# with the documented DMA-transpose ‖ DMA-copy xbar hazard.
BEST_CFG = dict(
    groups=(4, 4), xp_bufs=6, xtp_bufs=3, tr_split=8, phase_deps=True,
    pu_bufs=3, pcx_bufs=1, sp_bufs=4, up_bufs=6,
)


def kernel(inputs, mask, w, b, u):
    from concourse.bass_utils import run_bass_kernel_spmd

    nc = _get_nc(**BEST_CFG)
    x = np.ascontiguousarray(np.asarray(inputs, dtype=np.float32))
    m = np.ascontiguousarray(np.asarray(mask, dtype=np.int32))
    wf = np.ascontiguousarray(np.asarray(w, dtype=np.float32))
    bf = np.ascontiguousarray(np.asarray(b, dtype=np.float32))
    uf = np.ascontiguousarray(np.asarray(u, dtype=np.float32))

    in_maps = []
    for c in range(NCORES):
        sl = slice(c * EX, (c + 1) * EX)
        in_maps.append(
            {"x": x[sl], "mask": m[sl], "w": wf, "b": bf, "u": uf}
        )
    res = run_bass_kernel_spmd(nc, in_maps, core_ids=list(range(NCORES)))
    out = np.concatenate([res.results[c]["out"] for c in range(NCORES)], axis=0)
    return out.astype(np.float32)



# revision 5
# speedup vs baseline: 2.3012x; 2.3012x over previous
"""AttentionPooling Trainium2 Bass kernel, v2.

Problem (hardcoded shapes): B=64, T=4096, D=256, U=64
    uit    = tanh(inputs @ w + b)          # [B,T,U]
    scores = (uit @ u)[..., 0]             # [B,T]
    scores += (1-mask) * -1e9
    attn   = softmax(scores, axis=1)       # [B,T]
    out    = einsum('btd,bt->bd', inputs, attn)

Sharding: pure data-parallel, 8 examples per core across 8 NeuronCores.

v2 design (vs v1 baseline at 173.6us):
  - x loaded HBM->SBUF with f32->bf16 cast in FULL 256-elem rows (512B
    output descriptors -> full DMA bandwidth; v1's per-half loads had
    256B descriptors at half bandwidth)
  - NO xbar DMA transposes at all: every [128t,128d] tile of x is
    transposed on the TensorEngine (identity matmul, bf16 pass-through
    into PSUM), then copied PSUM->SBUF on DVE/ACT. Removes ~57us of
    serialized DMA-transpose time and the transpose/copy xbar hazard.
  - comp1/comp2/softmax structure unchanged from v1 (w stationary,
    z^T in PSUM, tanh+bias on ACT, block-diag u matmul, exp/mask/sum)
  - comp4 flipped: x2 [128t,128d] tiles are the STATIONARY operand and
    the attention column e[:,j] is the moving operand -> out free size 1
  - final scale via ones-row broadcast matmul + per-partition ACT scale
"""

import numpy as np

B, T, D, U = 64, 4096, 256, 64
NCORES = 8
EX = B // NCORES  # 8 examples per core
NJ = T // 128     # 32 T-chunks

_CACHE = {}


def _build(dve_banks=(0, 1, 2, 4, 6), tb_bufs=3, pz_bufs=2, ps_bufs=1,
           xp_bufs=8, xtp_bufs=2, up_bufs=4):
    """Build and compile the per-core Bass program."""
    import concourse.bacc as bacc
    import concourse.tile as tile
    import concourse.mybir as mybir
    from concourse.masks import make_identity
    from concourse._compat import axon_active

    f32 = mybir.dt.float32
    bf16 = mybir.dt.bfloat16
    i32 = mybir.dt.int32
    AF = mybir.ActivationFunctionType
    ALU = mybir.AluOpType

    nc = bacc.Bacc("TRN2", target_bir_lowering=False, debug=not axon_active())

    x_d = nc.dram_tensor("x", (EX, T, D), f32, kind="ExternalInput").ap()
    mask_d = nc.dram_tensor("mask", (EX, T), i32, kind="ExternalInput").ap()
    w_d = nc.dram_tensor("w", (D, U), f32, kind="ExternalInput").ap()
    b_d = nc.dram_tensor("b", (U,), f32, kind="ExternalInput").ap()
    u_d = nc.dram_tensor("u", (U, 1), f32, kind="ExternalInput").ap()
    out_d = nc.dram_tensor("out", (EX, D), f32, kind="ExternalOutput").ap()

    with tile.TileContext(nc) as tc:
        with (
            tc.tile_pool(name="consts", bufs=1) as cp,
            tc.tile_pool(name="xp", bufs=xp_bufs) as xp,
            tc.tile_pool(name="xtp", bufs=xtp_bufs) as xtp,
            tc.tile_pool(name="up", bufs=up_bufs) as up,
            tc.tile_pool(name="sp", bufs=4) as sp,
            tc.tile_pool(name="tb", bufs=tb_bufs, space="PSUM") as tbp,
            tc.tile_pool(name="pz", bufs=pz_bufs, space="PSUM") as pzp,
            tc.tile_pool(name="ps", bufs=ps_bufs, space="PSUM") as psp,
            tc.tile_pool(name="pc", bufs=2, space="PSUM") as pcp,
        ):
            # ---- first chunk of x(0) leads the Pool DGE queue; consts after ----
            ident = cp.tile([128, 128], bf16, tag="ident")
            make_identity(nc, ident)
            u_bd = cp.tile([128, 2], bf16, tag="u")
            nc.vector.memset(u_bd, 0.0)

            x2_0 = xp.tile([128, NJ, 256], bf16, tag="x2")
            for quar in range(2):
                nc.gpsimd.dma_start(
                    out=x2_0[:, 8 * quar : 8 * quar + 8, :],
                    in_=x_d[0, 1024 * quar : 1024 * quar + 1024].rearrange(
                        "(j p) d -> p j d", p=128
                    ),
                )
            w_bf = cp.tile([128, 2, U], bf16, tag="w")
            nc.gpsimd.dma_start(
                out=w_bf, in_=w_d.rearrange("(c p) u -> p c u", p=128)
            )
            nc.gpsimd.dma_start(out=u_bd[0:U, 0:1], in_=u_d)
            nc.gpsimd.dma_start(out=u_bd[U:128, 1:2], in_=u_d)
            b_sb = cp.tile([128, 1], f32, tag="b")
            b_2d = b_d.rearrange("(u o) -> u o", o=1)
            nc.sync.dma_start(out=b_sb[0:U, :], in_=b_2d)
            nc.sync.dma_start(out=b_sb[U:128, :], in_=b_2d)

            # mask: [2][128q, 128p] i32, q = 32*(ex%4)+j, p = t%128, L = ex//4
            mask3 = mask_d.rearrange("e t -> (e t)").rearrange(
                "(L q p) -> L q p", L=2, q=128
            )
            mb = []
            for L in range(2):
                mi_ = cp.tile([128, 128], i32, tag=f"mi{L}")
                nc.sync.dma_start(out=mi_, in_=mask3[L])
                mb_ = cp.tile([128, 128], bf16, tag=f"mb{L}")
                nc.vector.tensor_copy(out=mb_, in_=mi_)
                mb.append(mb_)
            mT = []
            for L in range(2):
                mT_ = cp.tile([128, 128], bf16, tag=f"mT{L}")
                mT.append(mT_)
            ones_b = cp.tile([128, 1], bf16, tag="ones")
            nc.vector.memset(ones_b, 1.0)
            ones_row = cp.tile([1, 128], f32, tag="onesr")
            nc.vector.memset(ones_row, 1.0)
            ctx_sb = cp.tile([128, EX, 2], f32, tag="ctx")

            # ---- remaining x loads upfront (merged 256-elem rows, SWDGE cast) ----
            x2s = [x2_0]
            for quar in range(2, 4):
                nc.gpsimd.dma_start(
                    out=x2_0[:, 8 * quar : 8 * quar + 8, :],
                    in_=x_d[0, 1024 * quar : 1024 * quar + 1024].rearrange(
                        "(j p) d -> p j d", p=128
                    ),
                )
            for ex in range(1, EX):
                x2 = xp.tile([128, NJ, 256], bf16, tag="x2")
                nc.gpsimd.dma_start(
                    out=x2, in_=x_d[ex].rearrange("(j p) d -> p j d", p=128)
                )
                x2s.append(x2)

            xTs = {}

            def emit_transpose_banks(ex, banks):
                """PE-transpose banks (4 j x 2 h each) of example ex and copy
                them into the xT tile. dve_banks picks the copy engine."""
                if ex not in xTs:
                    xT_new = xtp.tile([128, 2, NJ, 128], bf16, tag="xT")
                    xTs[ex] = xT_new
                xT = xTs[ex]
                x2 = x2s[ex]
                for bk in banks:
                    tb = tbp.tile([128, 8, 128], bf16, tag="tb")
                    for k in range(8):
                        j, h = 4 * bk + k // 2, k % 2
                        nc.tensor.transpose(
                            tb[:, k], x2[:, j, 128 * h : 128 * (h + 1)], ident
                        )
                    src = tb.rearrange("p (j h) d -> p h j d", h=2)
                    dst = xT[:, :, 4 * bk : 4 * bk + 4, :]
                    if bk in dve_banks:
                        nc.vector.tensor_copy(out=dst, in_=src)
                    else:
                        nc.scalar.activation(out=dst, in_=src, func=AF.Copy)

            def comp12(ex, es_h):
                """comp1 (z^T tiles) + tanh + comp2 (scores^T) + exp."""
                xT = xTs.pop(ex)
                ps = psp.tile([128, 33], f32, tag="ps")
                for Q in range(4):
                    H, gp = Q // 2, Q % 2
                    pz = pzp.tile([128, 512], f32, tag="pz")
                    for pi in range(2):
                        g = 2 * gp + pi
                        for h in range(2):
                            nc.tensor.matmul(
                                out=pz[64 * pi : 64 * pi + 64, :],
                                lhsT=w_bf[:, h, :],
                                rhs=xT[:, h, 16 * H + 4 * g : 16 * H + 4 * g + 4, :],
                                start=(h == 0),
                                stop=(h == 1),
                            )
                    uitQ = up.tile([128, 512], bf16, tag="uit")
                    nc.scalar.activation(
                        out=uitQ, in_=pz, func=AF.Tanh, bias=b_sb, scale=1.0
                    )
                    for cq in range(4):
                        # scores^T for both t-groups via block-diag u
                        nc.tensor.matmul(
                            out=ps[:, 16 * H + 8 * gp + cq : 16 * H + 8 * gp + cq + 5 : 4],
                            lhsT=uitQ[:, 128 * cq : 128 * cq + 128],
                            rhs=u_bd,
                            start=True,
                            stop=True,
                        )
                    if gp == 1:
                        # H-half of scores complete: exp right away so the
                        # ACT queue isn't stuck behind next-ex bank copies
                        es = sp.tile([128, 16], bf16, tag=f"es{H}")
                        nc.scalar.activation(
                            out=es, in_=ps[:, 16 * H : 16 * H + 16], func=AF.Exp
                        )
                        es_h.append(es)
                return ps

            def softmax(ex, ps, es_h):
                q0 = 32 * (ex % 4)
                e_h = []
                e1_h = []
                for Hh in range(2):
                    es = es_h[Hh]
                    e = sp.tile([128, 16], bf16, tag=f"e{Hh}")
                    nc.vector.tensor_mul(
                        e, es, mT[ex // 4][:, q0 + 16 * Hh : q0 + 16 * Hh + 16]
                    )
                    e1f = sp.tile([128, 1], f32, tag=f"e1f{Hh}")
                    nc.vector.tensor_reduce(
                        out=e1f, in_=e, axis=mybir.AxisListType.X, op=ALU.add
                    )
                    e_h.append(e)
                    e1_h.append(e1f)
                e1s = sp.tile([128, 1], f32, tag="e1s")
                nc.vector.tensor_add(e1s, e1_h[0], e1_h[1])
                e1b = sp.tile([128, 1], bf16, tag="e1b")
                nc.vector.tensor_copy(out=e1b, in_=e1s)
                # full denominator via ones-matmul -> ps[0:1, 32:33]
                nc.tensor.matmul(
                    out=ps[0:1, 32:33], lhsT=e1b, rhs=ones_b, start=True, stop=True
                )
                d_sb = sp.tile([1, 1], f32, tag="dsb")
                nc.vector.tensor_copy(out=d_sb, in_=ps[0:1, 32:33])
                return e_h, d_sb

            def comp4(ex, e_h, d_sb):
                x2 = x2s[ex]
                pcd = pcp.tile([128, 3], f32, tag="pcd")
                # one accumulation chain at a time: the PE does not support
                # two concurrently-open PSUM accumulation groups
                for h in range(2):
                    for j in range(NJ):
                        nc.tensor.matmul(
                            out=pcd[:, h : h + 1],
                            lhsT=x2[:, j, 128 * h : 128 * (h + 1)],
                            rhs=e_h[j // 16][:, j % 16 : j % 16 + 1],
                            start=(j == 0),
                            stop=(j == NJ - 1),
                            skip_group_check=True,
                        )
                # 1/denom broadcast to all partitions: ones_row^T @ d
                nc.tensor.matmul(
                    out=pcd[:, 2:3], lhsT=ones_row, rhs=d_sb,
                    start=True, stop=True, skip_group_check=True,
                )
                rb = sp.tile([128, 1], f32, tag="rb")
                nc.vector.reciprocal(out=rb, in_=pcd[:, 2:3])
                nc.vector.tensor_scalar_mul(
                    out=ctx_sb[:, ex, :], in0=pcd[:, 0:2], scalar1=rb
                )
                nc.sync.dma_start(
                    out=out_d[ex].rearrange("(h p) -> p h", p=128),
                    in_=ctx_sb[:, ex, :],
                )

            for ex in range(EX):
                if ex == 0:
                    emit_transpose_banks(0, range(8))
                es_h = []
                ps = comp12(ex, es_h)
                if ex == 0:
                    # mask transposes; needed first by softmax(0)
                    mtb = tbp.tile([128, 8, 128], bf16, tag="tb")
                    for L in range(2):
                        nc.tensor.transpose(mtb[:, L], mb[L], ident)
                    for L in range(2):
                        nc.vector.tensor_copy(out=mT[L], in_=mtb[:, L])
                e_h, d_sb = softmax(ex, ps, es_h)
                if ex + 1 < EX:
                    emit_transpose_banks(ex + 1, range(8))
                comp4(ex, e_h, d_sb)

    nc.compile()
    return nc


def _get_nc(**kw):
    key = tuple(sorted(kw.items()))
    if key not in _CACHE:
        _CACHE[key] = _build(**kw)
    return _CACHE[key]


BEST_CFG = dict(dve_banks=(0, 1, 2, 3, 4, 5, 6, 7))


def kernel(inputs, mask, w, b, u):
    from concourse.bass_utils import run_bass_kernel_spmd

    nc = _get_nc(**BEST_CFG)
    x = np.ascontiguousarray(np.asarray(inputs, dtype=np.float32))
    m = np.ascontiguousarray(np.asarray(mask, dtype=np.int32))
    wf = np.ascontiguousarray(np.asarray(w, dtype=np.float32))
    bf = np.ascontiguousarray(np.asarray(b, dtype=np.float32))
    uf = np.ascontiguousarray(np.asarray(u, dtype=np.float32))

    in_maps = []
    for c in range(NCORES):
        sl = slice(c * EX, (c + 1) * EX)
        in_maps.append(
            {"x": x[sl], "mask": m[sl], "w": wf, "b": bf, "u": uf}
        )
    res = run_bass_kernel_spmd(nc, in_maps, core_ids=list(range(NCORES)))
    out = np.concatenate([res.results[c]["out"] for c in range(NCORES)], axis=0)
    return out.astype(np.float32)


# revision 6
# speedup vs baseline: 2.3906x; 1.0388x over previous
"""AttentionPooling Trainium2 Bass kernel, v3.

Problem (hardcoded shapes): B=64, T=4096, D=256, U=64
    uit    = tanh(inputs @ w + b)          # [B,T,U]
    scores = (uit @ u)[..., 0]             # [B,T]
    scores += (1-mask) * -1e9
    attn   = softmax(scores, axis=1)       # [B,T]
    out    = einsum('btd,bt->bd', inputs, attn)

Sharding: pure data-parallel, 8 examples per core across 8 NeuronCores.

v3 over v2:
  - p-major token mapping t = 32*p + j: each SBUF partition p holds 32
    consecutive DRAM rows -> 128 16KB descriptors per example load
    (v2: 4096 512B descriptors), and the mask tiles load in exactly the
    scores layout (no mask transposes at all)
  - comp1 flipped: x^T tiles are the stationary operand, w the moving
    one -> out free size 64 (u) instead of 128 (t), halving comp1's
    PE row count; z lands in natural [t', j, u] layout
  - comp2 (u-weighted sum over u) leaves the PE: DVE tensor_mul with a
    broadcast u tile (2x bf16 mode) + grouped tensor_reduce on Pool/DVE
  - PSUM transposes staged in 2-bank tiles, copied out 2048 elems per
    instruction
  - b == 0 (per the problem spec fill) skips the bias entirely; a
    nonzero b builds a variant with a per-bank PSUM bias preload matmul
"""

import numpy as np

B, T, D, U = 64, 4096, 256, 64
NCORES = 8
EX = B // NCORES  # 8 examples per core
NJ = T // 128     # 32 j-chunks of 128 tokens

_CACHE = {}


def _build(with_bias=False, dve_units=(0, 2), mult_pool_exs=(0, 1, 2, 3, 4, 5),
           xp_bufs=8, xtp_bufs=2, uit_bufs=2, pz_bufs=3, per_ex_scores=True,
           comp2_pe=False, pool_units=()):
    """Build and compile the per-core Bass program."""
    import concourse.bacc as bacc
    import concourse.tile as tile
    import concourse.mybir as mybir
    from concourse.masks import make_identity
    from concourse._compat import axon_active

    f32 = mybir.dt.float32
    bf16 = mybir.dt.bfloat16
    i32 = mybir.dt.int32
    AF = mybir.ActivationFunctionType
    ALU = mybir.AluOpType

    nc = bacc.Bacc("TRN2", target_bir_lowering=False, debug=not axon_active())

    x_d = nc.dram_tensor("x", (EX, T, D), f32, kind="ExternalInput").ap()
    mask_d = nc.dram_tensor("mask", (EX, T), i32, kind="ExternalInput").ap()
    w_d = nc.dram_tensor("w", (D, U), f32, kind="ExternalInput").ap()
    b_d = nc.dram_tensor("b", (U,), f32, kind="ExternalInput").ap()
    u_d = nc.dram_tensor("u", (U, 1), f32, kind="ExternalInput").ap()
    out_d = nc.dram_tensor("out", (EX, D), f32, kind="ExternalOutput").ap()

    with tile.TileContext(nc) as tc:
        with (
            tc.tile_pool(name="consts", bufs=1) as cp,
            tc.tile_pool(name="xp", bufs=xp_bufs) as xp,
            tc.tile_pool(name="xtp", bufs=xtp_bufs) as xtp,
            tc.tile_pool(name="up", bufs=uit_bufs) as up,
            tc.tile_pool(name="sp", bufs=4) as sp,
            tc.tile_pool(name="tb", bufs=2, space="PSUM") as tbp,
            tc.tile_pool(name="pz", bufs=pz_bufs, space="PSUM") as pzp,
            tc.tile_pool(name="ps", bufs=1, space="PSUM") as psp,
            tc.tile_pool(name="pc", bufs=1, space="PSUM") as pcp,
        ):
            # ---- first chunks of x(0) lead the Pool DGE queue ----
            ident = cp.tile([128, 128], bf16, tag="ident")
            make_identity(nc, ident)
            u_bd = cp.tile([1, U], f32, tag="u")

            xv = x_d.rearrange("e (p j) d -> e p j d", j=NJ)
            x2_0 = xp.tile([128, NJ, 256], bf16, tag="x2")
            for quar in range(2):
                nc.gpsimd.dma_start(
                    out=x2_0[:, 8 * quar : 8 * quar + 8, :],
                    in_=xv[0, :, 8 * quar : 8 * quar + 8, :],
                )
            w_bf = cp.tile([128, 2, U], bf16, tag="w")
            nc.gpsimd.dma_start(
                out=w_bf, in_=w_d.rearrange("(c p) u -> p c u", p=128)
            )
            nc.sync.dma_start(out=u_bd, in_=u_d.rearrange("u o -> o u"))
            for quar in range(2, 4):
                nc.gpsimd.dma_start(
                    out=x2_0[:, 8 * quar : 8 * quar + 8, :],
                    in_=xv[0, :, 8 * quar : 8 * quar + 8, :],
                )

            # u broadcast to all partitions (ones-column matmul), cast bf16
            ones_row = cp.tile([1, 128], f32, tag="onesr")
            nc.vector.memset(ones_row, 1.0)
            ub_ps = pzp.tile([128, U], f32, tag="pz")
            nc.tensor.matmul(
                out=ub_ps, lhsT=ones_row, rhs=u_bd, start=True, stop=True
            )
            ub = cp.tile([128, U], bf16, tag="ub")
            nc.vector.tensor_copy(out=ub, in_=ub_ps)
            if comp2_pe:
                # block-diagonal u for the packed (2j x 64u) uitT matmul
                u_bd = cp.tile([128, 2], bf16, tag="ubd")
                nc.vector.memset(u_bd, 0.0)
                nc.gpsimd.dma_start(out=u_bd[0:U, 0:1], in_=u_d)
                nc.gpsimd.dma_start(out=u_bd[U:128, 1:2], in_=u_d)

            if with_bias:
                b_rep = cp.tile([1, 8, U], f32, tag="brep")
                for g8 in range(8):
                    nc.sync.dma_start(
                        out=b_rep[:, g8, :], in_=b_d.rearrange("(o u) -> o u", o=1)
                    )
                b_rep_bf = cp.tile([1, 8, U], bf16, tag="brepbf")
                nc.vector.tensor_copy(out=b_rep_bf, in_=b_rep)
                ones_row_bf = cp.tile([1, 128], bf16, tag="onesrb")
                nc.vector.memset(ones_row_bf, 1.0)

            # masks: p-major layout needs no transpose at all
            mbs = []
            for ex in range(EX):
                mi_ = cp.tile([128, NJ], i32, tag=f"mi{ex}")
                nc.sync.dma_start(
                    out=mi_, in_=mask_d[ex].rearrange("(p j) -> p j", j=NJ)
                )
                mb_ = cp.tile([128, NJ], bf16, tag=f"mb{ex}")
                nc.vector.tensor_copy(out=mb_, in_=mi_)
                mbs.append(mb_)

            ones_b = cp.tile([128, 1], bf16, tag="ones")
            nc.vector.memset(ones_b, 1.0)
            ctx_sb = cp.tile([128, EX, 2], f32, tag="ctx")

            # ---- remaining x loads upfront ----
            x2s = [x2_0]
            for ex in range(1, EX):
                x2 = xp.tile([128, NJ, 256], bf16, tag="x2")
                nc.gpsimd.dma_start(out=x2, in_=xv[ex])
                x2s.append(x2)

            xTs = {}

            def emit_transpose_units(ex, units):
                """PE-transpose 2-bank units (8 j x 2 h) of example ex and
                copy them into the xT tile; dve_units picks the engine."""
                if ex not in xTs:
                    xT_new = xtp.tile([128, 2, NJ, 128], bf16, tag="xT")
                    xTs[ex] = xT_new
                xT = xTs[ex]
                x2 = x2s[ex]
                for un in units:
                    tb = tbp.tile([128, 16, 128], bf16, tag="tb")
                    for k in range(16):
                        j, h = 8 * un + k // 2, k % 2
                        nc.tensor.transpose(
                            tb[:, k], x2[:, j, 128 * h : 128 * (h + 1)], ident
                        )
                    src = tb.rearrange("p (j h) d -> p h j d", h=2)
                    dst = xT[:, :, 8 * un : 8 * un + 8, :]
                    if un in dve_units:
                        nc.vector.tensor_copy(out=dst, in_=src)
                    elif un in pool_units:
                        nc.gpsimd.tensor_copy(out=dst, in_=src)
                    else:
                        nc.scalar.activation(out=dst, in_=src, func=AF.Copy)

            def comp12(ex, next_ex=None):
                """comp1 flipped (z natural) + tanh + comp2 (either packed
                uitT matmuls on PE, or u-mult + grouped reduce on DVE)."""
                xT = xTs.pop(ex)
                uit = up.tile([128, NJ, U], bf16, tag="uit")
                if comp2_pe:
                    uitT_sb = sp.tile([128, 16, 128], bf16, tag="uitT", bufs=2)
                    # scores live in cols 4:36 of the ctx/denom psum bank
                    ps = pcp.tile([128, 36], f32, tag="pcd")
                    scores_sb = None
                    ut_tiles = {}
                else:
                    scores_sb = sp.tile([128, NJ, 1], f32, tag="scores")
                mul_eng = nc.gpsimd if ex in mult_pool_exs else nc.vector

                def emit_pairs(g):
                    # PE-transpose 4 packed (2j x 64u) uit pair tiles of
                    # bank g into a single-bank psum unit (half the pairs)
                    half = g // 2
                    if half not in ut_tiles:
                        ut_new = psp.tile([128, 8, 128], bf16, tag="ut")
                        ut_tiles[half] = ut_new
                    ut = ut_tiles[half]
                    for pr in range(4 * g, 4 * g + 4):
                        nc.tensor.transpose(
                            ut[:, pr % 8],
                            uit[:, 2 * pr : 2 * pr + 2, :].rearrange(
                                "p j u -> p (j u)"
                            ),
                            ident,
                        )

                for g in range(4):
                    pz = pzp.tile([128, 8, U], f32, tag="pz")
                    if with_bias:
                        nc.tensor.matmul(
                            out=pz.rearrange("p j u -> p (j u)"),
                            lhsT=ones_row_bf,
                            rhs=b_rep_bf.rearrange("o j u -> o (j u)"),
                            start=True,
                            stop=False,
                            skip_group_check=True,
                        )
                    for jj in range(8):
                        j = 8 * g + jj
                        for h in range(2):
                            nc.tensor.matmul(
                                out=pz[:, jj, :],
                                lhsT=xT[:, h, j, :],
                                rhs=w_bf[:, h, :],
                                start=(h == 0 and not with_bias),
                                stop=(h == 1),
                                skip_group_check=with_bias,
                            )
                    nc.scalar.activation(
                        out=uit[:, 8 * g : 8 * g + 8, :], in_=pz, func=AF.Tanh
                    )
                    if comp2_pe:
                        # pairs of bank g-1 (tanh g-1 has drained by now)
                        if g >= 1:
                            emit_pairs(g - 1)
                        if g == 2:
                            nc.vector.tensor_copy(
                                out=uitT_sb[:, 0:8], in_=ut_tiles[0]
                            )
                        continue
                    if not per_ex_scores:
                        uu = sp.tile([128, 8, U], bf16, tag="uu")
                        mul_eng.tensor_mul(
                            uu,
                            uit[:, 8 * g : 8 * g + 8, :],
                            ub[:, None, :].broadcast_to([128, 8, U]),
                        )
                        nc.vector.tensor_reduce(
                            out=scores_sb[:, 8 * g : 8 * g + 8, :],
                            in_=uu,
                            axis=mybir.AxisListType.X,
                            op=ALU.add,
                        )
                if comp2_pe:
                    emit_pairs(3)
                    nc.vector.tensor_copy(
                        out=uitT_sb[:, 8:16], in_=ut_tiles[1]
                    )
                    if next_ex is not None:
                        emit_transpose_units(next_ex, (0,))
                    for pr in range(16):
                        nc.tensor.matmul(
                            out=ps[:, 4 + 2 * pr : 4 + 2 * pr + 2],
                            lhsT=uitT_sb[:, pr],
                            rhs=u_bd,
                            start=True,
                            stop=True,
                            skip_group_check=True,
                        )
                    return ps
                if per_ex_scores:
                    uu = sp.tile([128, NJ, U], bf16, tag="uu")
                    mul_eng.tensor_mul(
                        uu, uit, ub[:, None, :].broadcast_to([128, NJ, U])
                    )
                    nc.vector.tensor_reduce(
                        out=scores_sb,
                        in_=uu,
                        axis=mybir.AxisListType.X,
                        op=ALU.add,
                    )
                return scores_sb

            def softmax(ex, scores):
                es = sp.tile([128, NJ], bf16, tag="es")
                if comp2_pe:
                    sc_view = scores[:, 4 : 4 + NJ]
                else:
                    sc_view = scores.rearrange("p j o -> p (j o)")
                nc.scalar.activation(out=es, in_=sc_view, func=AF.Exp)
                e = sp.tile([128, NJ], bf16, tag="e")
                nc.vector.tensor_mul(e, es, mbs[ex])
                e1f = sp.tile([128, 1], f32, tag="e1f")
                nc.vector.tensor_reduce(
                    out=e1f, in_=e, axis=mybir.AxisListType.X, op=ALU.add
                )
                e1b = sp.tile([128, 1], bf16, tag="e1b")
                nc.vector.tensor_copy(out=e1b, in_=e1f)
                return e, e1b

            def comp4(ex, e, e1b, pcd=None):
                x2 = x2s[ex]
                if pcd is None:
                    pcd = pcp.tile([128, 4], f32, tag="pcd")
                # one accumulation chain at a time: the PE does not support
                # two concurrently-open PSUM accumulation groups
                for h in range(2):
                    for j in range(NJ):
                        nc.tensor.matmul(
                            out=pcd[:, h : h + 1],
                            lhsT=x2[:, j, 128 * h : 128 * (h + 1)],
                            rhs=e[:, j : j + 1],
                            start=(j == 0),
                            stop=(j == NJ - 1),
                            skip_group_check=True,
                        )
                # denominator, then broadcast to all partitions
                nc.tensor.matmul(
                    out=pcd[0:1, 2:3], lhsT=e1b, rhs=ones_b,
                    start=True, stop=True, skip_group_check=True,
                )
                d_sb = sp.tile([1, 1], f32, tag="dsb")
                nc.vector.tensor_copy(out=d_sb, in_=pcd[0:1, 2:3])
                nc.tensor.matmul(
                    out=pcd[:, 3:4], lhsT=ones_row, rhs=d_sb,
                    start=True, stop=True, skip_group_check=True,
                )
                rb = sp.tile([128, 1], f32, tag="rb")
                nc.vector.reciprocal(out=rb, in_=pcd[:, 3:4])
                nc.vector.tensor_scalar_mul(
                    out=ctx_sb[:, ex, :], in0=pcd[:, 0:2], scalar1=rb
                )
                nc.sync.dma_start(
                    out=out_d[ex].rearrange("(h p) -> p h", p=128),
                    in_=ctx_sb[:, ex, :],
                )

            for ex in range(EX):
                if ex == 0:
                    emit_transpose_units(0, range(4))
                nxt = ex + 1 if ex + 1 < EX else None
                scores = comp12(ex, nxt)
                e, e1b = softmax(ex, scores)
                if nxt is not None:
                    emit_transpose_units(nxt, (1, 2, 3) if comp2_pe else range(4))
                comp4(ex, e, e1b, scores if comp2_pe else None)

    nc.compile()
    return nc


def _get_nc(**kw):
    key = tuple(sorted(kw.items()))
    if key not in _CACHE:
        _CACHE[key] = _build(**kw)
    return _CACHE[key]


BEST_CFG = dict(per_ex_scores=False, mult_pool_exs=(), pz_bufs=2,
                uit_bufs=3, xtp_bufs=3, dve_units=(0, 2))


def kernel(inputs, mask, w, b, u):
    from concourse.bass_utils import run_bass_kernel_spmd

    x = np.ascontiguousarray(np.asarray(inputs, dtype=np.float32))
    m = np.ascontiguousarray(np.asarray(mask, dtype=np.int32))
    wf = np.ascontiguousarray(np.asarray(w, dtype=np.float32))
    bf = np.ascontiguousarray(np.asarray(b, dtype=np.float32))
    uf = np.ascontiguousarray(np.asarray(u, dtype=np.float32))

    cfg = dict(BEST_CFG)
    cfg["with_bias"] = bool(np.any(bf))
    nc = _get_nc(**cfg)

    in_maps = []
    for c in range(NCORES):
        sl = slice(c * EX, (c + 1) * EX)
        in_maps.append(
            {"x": x[sl], "mask": m[sl], "w": wf, "b": bf, "u": uf}
        )
    res = run_bass_kernel_spmd(nc, in_maps, core_ids=list(range(NCORES)))
    out = np.concatenate([res.results[c]["out"] for c in range(NCORES)], axis=0)
    return out.astype(np.float32)
